# revision 2
# baseline (speedup 1.0000x reference)
"""AG-GEMM on 8 TRN2 NeuronCores — 2-level Strassen-Winograd on a 4x2 core grid.

Reference: A_full[8192, 4096] @ weight.T[4096, 4096] -> C[8192, 4096].

Core r = (mg, ng) with mg = r % 4, ng = r // 4 computes the output slice
C[mg*2048:(mg+1)*2048, ng*2048:(ng+1)*2048] as CT = P @ Q with
P = W_rows[ng] [2048 n, 4096 k] and Q = A_rows[mg]^T [4096 k, 2048 m]
(weight row-slices replicated across the 4 cores of an n-group; no
collective). The 4x2 grid keeps per-core M at 2048 so that after TWO
Strassen-Winograd levels the matmul moving-dim is still 512 (the PSUM-bank
sweet spot). 49 quarter-products [512n,1024k]@[1024k,512m] replace 64:
PE work drops to 49/64 of dense (1568 vs 2048 128x512 matmuls).

All operand-side combinations (both levels) are precomputed on the host.
The device runs the 49 GEMMs plus the two-level output recombination:
inner (L2) U-terms on DVE fused with the PSUM drains, outer (L1) U-terms
on GpSimd from fp16 staging. Staging is fp16 (error budget allows it),
which halves both DVE time and SBUF footprint; output is written fp16 and
upcast on the host.

Winograd scheme per level (products executed M1,M6,M2,M7,M4,M5,M3):
  M1=P11*Q11  M2=P12*Q21  M3=S4*Q22  M4=P22*T4  M5=S1*T1  M6=S2*T2  M7=S3*T3
  C11=M1+M2  U2=M1+M6  U3=U2+M7  U4=U2+M5  C12=U4+M3  C21=U3-M4  C22=U3+M5
The exec order keeps at most 3 staging groups alive per level:
L1 24 x [128,1024] fp16 tiles (48 KB/part), L2 2x12 x [128,512] (24 KB/part).

Per product one 1 MB W blob (ACT HWDGE ring) and one 1 MB Q blob (SP ring),
both contiguous 8 KB/partition, prefetched 2 products ahead; out pieces
(128 KB) ride the SP ring as produced. Product 0's W is host-packed
kt-major and interleaved with its Q chunks in exact consumption order so
the PE tracks the DMA arrival frontier during the ramp (first 2 layers on
the otherwise-idle ACT ring); its 4 column sweeps run kt-interleaved
across 4 PSUM banks. The final sweep runs as two m-halves so half X's
drain hides under half Y's matmuls. PSUM banks rotate mod 8; every bank
is drained by the fused DVE combo right after its accumulation stops.
"""

import numpy as np

WORLD = 8
PM, PN = 4, 2          # core grid: 4 M-groups x 2 N-groups
MC = 2048              # per-core M (= rows of C slice)
NCOLS = 2048           # per-core N (= cols of C slice)
K = 4096
NPROD = 49
EXEC = [1, 6, 2, 7, 4, 5, 3]   # Winograd product execution order (both levels)

MM_DTYPE = "float16"


def _lcombos(P):
    # left/weight-side combos of [n, k] matrix quadrants
    n, k = P.shape
    h, g = n // 2, k // 2
    P11, P12 = P[:h, :g], P[:h, g:]
    P21, P22 = P[h:, :g], P[h:, g:]
    S1 = P21 + P22
    S2 = S1 - P11
    S3 = P11 - P21
    S4 = P12 - S2
    return {1: P11, 2: P12, 3: S4, 4: P22, 5: S1, 6: S2, 7: S3}


def _rcombos_mT(Am):
    # right-side combos, m-major (Am = R^T [m, k]); returns RT_p [m/2, k/2]
    m, k = Am.shape
    mh, kh = m // 2, k // 2
    B00, B01 = Am[:mh, :kh], Am[:mh, kh:]
    B10, B11 = Am[mh:, :kh], Am[mh:, kh:]
    return {
        1: B00,
        2: B01,
        3: B11,
        4: B11 - B10 + B00 - B01,
        5: B10 - B00,
        6: B11 - B10 + B00,
        7: B11 - B10,
    }


def _build_nc():
    from contextlib import ExitStack

    from concourse import bacc, mybir, tile

    f32 = mybir.dt.float32
    mm_dt = getattr(mybir.dt, MM_DTYPE)

    nc = bacc.Bacc("TRN2", target_bir_lowering=False, debug=False)

    # product 0 W, kt-major: w1[kp, kt*512 + c*128 + j] = L0[128c+j, 128kt+kp]
    w1_ext = nc.dram_tensor("w1", [128, 4096], mm_dt, kind="ExternalInput")
    # products 1..48, c-major: w[i, kp, c*1024 + kt*128 + j] = L[128c+j, 128kt+kp]
    w_ext = nc.dram_tensor("w", [48, 128, 4096], mm_dt, kind="ExternalInput")
    # rhs blobs: a[i, kp, kt*512 + m] = R[128kt+kp, m]
    a_ext = nc.dram_tensor("a", [NPROD, 128, 4096], mm_dt, kind="ExternalInput")
    # out rows: row = b*16 + t*2 + mh; b in (C11,C12,C21,C22), t n-tile, mh m-half
    out_ext = nc.dram_tensor("out", [64, 128, 512], mm_dt, kind="ExternalOutput")

    with tile.TileContext(nc) as tc, ExitStack() as ctx:
        q_pool = ctx.enter_context(tc.tile_pool(name="q", bufs=4))
        w_pool = ctx.enter_context(tc.tile_pool(name="w", bufs=4))
        w1_pool = ctx.enter_context(tc.tile_pool(name="w1", bufs=1))
        l1_pool = ctx.enter_context(tc.tile_pool(name="l1", bufs=1))
        l2_pool = ctx.enter_context(tc.tile_pool(name="l2", bufs=1))
        mt_pool = ctx.enter_context(tc.tile_pool(name="mt", bufs=8))
        o_pool = ctx.enter_context(tc.tile_pool(name="o", bufs=8))
        s_pool = ctx.enter_context(tc.tile_pool(name="s", bufs=1))
        ps_pool = ctx.enter_context(tc.tile_pool(name="ps", bufs=1, space="PSUM"))

        # ---- PE warmup during the DMA ramp (pulls the HAM clock-gate) ----
        wu = s_pool.tile([128, 128], mm_dt, name="wu", tag="wu")
        nc.vector.memset(wu[:], 0)
        ps_wu = ps_pool.tile([128, 512], f32, name="pswu", tag="b6")
        for _ in range(32):
            nc.tensor.matmul(ps_wu[:, :128], wu[:], wu[:], start=True, stop=True)

        # ---- ramp: product 0's W layers + Q chunks in consumption order ----
        q0 = q_pool.tile([128, 4096], mm_dt, name="q0", tag="q")
        w1 = w1_pool.tile([128, 4096], mm_dt, name="w1", tag="w1")
        # w1 rides the ACT ring in kt-layer pairs; q0 rides SP in kt chunks —
        # both in consumption order, so the two rings split the ramp evenly
        nc.scalar.dma_start(w1[:, 0:1024], w1_ext[:, 0:1024])     # kt layers 0-1
        nc.sync.dma_start(q0[:, 0:512], a_ext[0, :, 0:512])
        nc.sync.dma_start(q0[:, 512:1024], a_ext[0, :, 512:1024])
        for t in range(1, 4):
            nc.scalar.dma_start(
                w1[:, t * 1024 : (t + 1) * 1024], w1_ext[:, t * 1024 : (t + 1) * 1024]
            )
            nc.sync.dma_start(
                q0[:, 2 * t * 512 : (2 * t + 1) * 512],
                a_ext[0, :, 2 * t * 512 : (2 * t + 1) * 512],
            )
            nc.sync.dma_start(
                q0[:, (2 * t + 1) * 512 : (2 * t + 2) * 512],
                a_ext[0, :, (2 * t + 1) * 512 : (2 * t + 2) * 512],
            )

        # ---- streamed W/Q blobs, 2 products ahead ----
        w_bufs, q_bufs = [], []

        def issue_w(idx):
            if idx >= NPROD:
                return
            wt = w_pool.tile([128, 4096], mm_dt, name=f"w{idx}", tag="w")
            nc.scalar.dma_start(wt[:], w_ext[idx - 1])
            w_bufs.append(wt)

        def issue_q(idx):
            if idx >= NPROD:
                return
            qt = q_pool.tile([128, 4096], mm_dt, name=f"q{idx}", tag="q")
            nc.sync.dma_start(qt[:], a_ext[idx])
            q_bufs.append(qt)

        for idx in (1, 2):
            issue_w(idx)
            issue_q(idx)

        # ---- staging ----
        groups = {}  # logical L1 group name -> 8 tiles [128, 1024]

        def new_group(name, tag):
            groups[name] = [
                l1_pool.tile(
                    [128, 1024], mm_dt, name=f"{name}_{t}", tag=f"L{tag}{t}"
                )
                for t in range(8)
            ]

        def l2_group(i, g):
            return [
                l2_pool.tile(
                    [128, 512], mm_dt, name=f"i{i}{g}{c}", tag=f"i{i % 2}{g}{c}"
                )
                for c in range(4)
            ]

        sweep = 0
        m1q = u2q = u3q = None
        u4q = [None] * 4

        def out_piece(i, op, l1t, dst_sl, b, t, mh, mlo, mhi, tmp):
            o = o_pool.tile(
                [128, mhi - mlo], mm_dt, name=f"o{i}_{b}_{t}_{mh}", tag="o"
            )
            g_op = nc.vector.tensor_add if op == "add" else nc.vector.tensor_sub
            g_op(o[:], l1t[:, dst_sl], tmp[:])
            nc.sync.dma_start(out_ext[b * 16 + t * 2 + mh, :, mlo:mhi], o[:])

        def finish(i, c, ps, op, stage_tile, t, mh, mlo, mhi):
            # inner-final piece (quadrant of outer product Mp) + L1-level op
            msl = slice(mlo, mhi)
            dst_sl = slice(mh * 512 + mlo, mh * 512 + mhi)
            vec_op = nc.vector.tensor_add if op == "add" else nc.vector.tensor_sub
            if i == 0:
                vec_op(groups["M1"][t][:, dst_sl], stage_tile[:, msl], ps[:])
                return
            tmp = mt_pool.tile(
                [128, mhi - mlo], mm_dt, name=f"mt{i}_{t}_{mh}", tag="mt"
            )
            vec_op(tmp[:], stage_tile[:, msl], ps[:])
            p = EXEC[i]
            if p == 6:    # U2 = M1 + M6
                nc.vector.tensor_add(
                    groups["U2"][t][:, dst_sl], groups["M1"][t][:, dst_sl], tmp[:]
                )
            elif p == 7:  # U3 = U2 + M7
                nc.vector.tensor_add(
                    groups["U3"][t][:, dst_sl], groups["U2"][t][:, dst_sl], tmp[:]
                )
            elif p == 5:  # U4 = U2 + M5 ; C22 = U3 + M5
                nc.vector.tensor_add(
                    groups["U4"][t][:, dst_sl], groups["U2"][t][:, dst_sl], tmp[:]
                )
                out_piece(i, "add", groups["U3"][t], dst_sl, 3, t, mh, mlo, mhi, tmp)
            elif p == 2:  # C11 = M1 + M2
                out_piece(i, "add", groups["M1"][t], dst_sl, 0, t, mh, mlo, mhi, tmp)
            elif p == 4:  # C21 = U3 - M4
                out_piece(i, "sub", groups["U3"][t], dst_sl, 2, t, mh, mlo, mhi, tmp)
            else:         # p == 3: C12 = U4 + M3
                out_piece(i, "add", groups["U4"][t], dst_sl, 1, t, mh, mlo, mhi, tmp)

        def do_combo(i, j, c, ps, mlo=0, mhi=512):
            msl = slice(mlo, mhi)
            if j == 0:    # m1 of this outer product
                nc.vector.tensor_copy(m1q[c][:, msl], ps[:])
            elif j == 1:  # q=6: u2 = m1 + m6
                nc.vector.tensor_add(u2q[c][:, msl], m1q[c][:, msl], ps[:])
            elif j == 3:  # q=7: u3 = u2 + m7
                nc.vector.tensor_add(u3q[c][:, msl], u2q[c][:, msl], ps[:])
            elif j == 2:  # q=2: quadrant 11 = m1 + m2
                finish(i, c, ps, "add", m1q[c], c, 0, mlo, mhi)
            elif j == 4:  # q=4: quadrant 21 = u3 - m4
                finish(i, c, ps, "sub", u3q[c], 4 + c, 0, mlo, mhi)
            elif j == 5:  # q=5: u4 = u2 + m5 ; quadrant 22 = u3 + m5
                nc.vector.tensor_add(u4q[c][:, msl], u2q[c][:, msl], ps[:])
                finish(i, c, ps, "add", u3q[c], 4 + c, 1, mlo, mhi)
            else:         # j=6, q=3: quadrant 12 = u4 + m3
                finish(i, c, ps, "add", u4q[c], c, 1, mlo, mhi)

        # ---- main loop: 49 products ----
        for i in range(7):
            if i == 0:
                new_group("M1", "A")
            elif i == 1:
                new_group("U2", "B")
            elif i == 3:
                new_group("U3", "C")
            elif i == 5:
                new_group("U4", "A")
            for j in range(7):
                idx = 7 * i + j
                if j == 0:
                    m1q = l2_group(i, "A")
                elif j == 1:
                    u2q = l2_group(i, "B")
                elif j == 3:
                    u3q = l2_group(i, "C")
                elif j == 5:
                    u4q = l2_group(i, "D")

                if idx == 0:
                    # kt-interleaved across 4 banks: PE tracks the DMA frontier
                    psA = [
                        ps_pool.tile([128, 512], f32, name=f"psA{c}", tag=f"b{c}")
                        for c in range(4)
                    ]
                    sweep = 4
                    for kt in range(8):
                        for c in range(4):
                            nc.tensor.matmul(
                                psA[c][:],
                                w1[:, kt * 512 + c * 128 : kt * 512 + (c + 1) * 128],
                                q0[:, kt * 512 : (kt + 1) * 512],
                                start=(kt == 0),
                                stop=(kt == 7),
                            )
                    for c in range(4):
                        do_combo(i, j, c, psA[c])
                    continue

                qt, wt = q_bufs.pop(0), w_bufs.pop(0)
                issue_w(idx + 2)
                issue_q(idx + 2)
                for c in range(4):
                    spans = [(0, 512)]
                    if idx == NPROD - 1 and c == 3:
                        # split the final sweep into m-halves to hide the drain
                        spans = [(0, 256), (256, 512)]
                    for mlo, mhi in spans:
                        ps = ps_pool.tile(
                            [128, mhi - mlo],
                            f32,
                            name=f"ps{idx}_{c}_{mlo}",
                            tag=f"b{sweep % 8}",
                        )
                        sweep += 1
                        for kt in range(8):
                            nc.tensor.matmul(
                                ps[:],
                                wt[:, c * 1024 + kt * 128 : c * 1024 + (kt + 1) * 128],
                                qt[:, kt * 512 + mlo : kt * 512 + mhi],
                                start=(kt == 0),
                                stop=(kt == 7),
                            )
                        do_combo(i, j, c, ps, mlo, mhi)

    nc.compile()
    return nc


def _prep_inputs(A_shards, weight, transed_weight=0):
    np_dt = np.float16

    try:
        transed = bool(int(np.asarray(transed_weight)))
    except (TypeError, ValueError):
        transed = bool(transed_weight)

    Wf = np.asarray(weight, dtype=np.float32)
    Wn = Wf.T if transed else Wf            # [N, K], rows = output columns
    A = np.asarray(A_shards, dtype=np.float32).reshape(WORLD * 1024, K)

    w_blobs = []
    for ng in range(PN):
        P = Wn[ng * NCOLS : (ng + 1) * NCOLS]      # [2048, 4096]
        Lout = _lcombos(P)
        w1 = None
        wrest = np.empty((48, 128, 4096), np_dt)
        for i, p in enumerate(EXEC):
            Lin = _lcombos(Lout[p])
            for j, q in enumerate(EXEC):
                idx = 7 * i + j
                Lc = Lin[q].astype(np_dt)          # [512, 1024]
                t4 = Lc.reshape(4, 128, 8, 128)    # [c, j, kt, kp]
                if idx == 0:
                    w1 = np.ascontiguousarray(
                        t4.transpose(3, 2, 0, 1).reshape(128, 4096)
                    )
                else:
                    wrest[idx - 1] = t4.transpose(3, 0, 2, 1).reshape(128, 4096)
        w_blobs.append((w1, wrest))

    in_maps = []
    for r in range(WORLD):
        mg, ng = r % PM, r // PM
        Am = A[mg * MC : (mg + 1) * MC]            # [2048, 4096] = R^T m-major
        Rout = _rcombos_mT(Am)
        a_blob = np.empty((NPROD, 128, 4096), np_dt)
        for i, p in enumerate(EXEC):
            Rin = _rcombos_mT(Rout[p])
            for j, q in enumerate(EXEC):
                RTc = Rin[q].astype(np_dt)         # [512 m, 1024 k]
                a_blob[7 * i + j] = (
                    RTc.T.reshape(8, 128, 512).transpose(1, 0, 2).reshape(128, 4096)
                )
        w1, wrest = w_blobs[ng]
        in_maps.append({"w1": w1, "w": wrest, "a": a_blob})
    return in_maps


def _gather_output(results):
    C = np.empty((WORLD * 1024, K), np.float32)
    for r in range(WORLD):
        mg, ng = r % PM, r // PM
        o = np.asarray(results[r]["out"], dtype=np.float32).reshape(4, 8, 2, 128, 512)
        CT = np.empty((NCOLS, MC), np.float32)
        for b, (n0, m0) in enumerate([(0, 0), (0, 1024), (1024, 0), (1024, 1024)]):
            # o[b]: [nt, mh, p, m] -> block[n = nt*128+p, m = mh*512+m]
            CT[n0 : n0 + 1024, m0 : m0 + 1024] = (
                o[b].transpose(0, 2, 1, 3).reshape(1024, 1024)
            )
        C[mg * MC : (mg + 1) * MC, ng * NCOLS : (ng + 1) * NCOLS] = CT.T
    return C


_NC = None


def _get_nc():
    global _NC
    if _NC is None:
        _NC = _build_nc()
    return _NC


def kernel(A_shards, weight, transed_weight=0, **_ignored):
    from concourse import bass_utils

    nc = _get_nc()
    in_maps = _prep_inputs(A_shards, weight, transed_weight)
    res = bass_utils.run_bass_kernel_spmd(nc, in_maps, core_ids=list(range(WORLD)))
    return _gather_output(res.results)


if __name__ == "__main__":
    rng = np.random.default_rng(0)
    A = rng.standard_normal((WORLD, 1024, K), dtype=np.float32)
    W = (rng.standard_normal((K, K), dtype=np.float32) * 0.02).astype(np.float32)
    out = kernel(A, W, 0)
    ref = A.reshape(WORLD * 1024, K) @ W.T
    err = np.abs(out - ref).max() / max(np.abs(ref).max(), 1e-12)
    print("abs-rel err vs local numpy:", err)


# revision 4
# speedup vs baseline: 1.0010x; 1.0010x over previous
"""AG-GEMM on 8 TRN2 NeuronCores — 2-level Strassen-Winograd on a 4x2 core grid.

Reference: A_full[8192, 4096] @ weight.T[4096, 4096] -> C[8192, 4096].

Core r = (mg, ng) with mg = r % 4, ng = r // 4 computes the output slice
C[mg*2048:(mg+1)*2048, ng*2048:(ng+1)*2048] as CT = P @ Q with
P = W_rows[ng] [2048 n, 4096 k] and Q = A_rows[mg]^T [4096 k, 2048 m]
(weight row-slices replicated across the 4 cores of an n-group; no
collective). The 4x2 grid keeps per-core M at 2048 so that after TWO
Strassen-Winograd levels the matmul moving-dim is still 512 (the PSUM-bank
sweet spot). 49 quarter-products [512n,1024k]@[1024k,512m] replace 64:
PE work drops to 49/64 of dense (1568 vs 2048 128x512 matmuls).

All operand-side combinations (both levels) are precomputed on the host.
The device runs the 49 GEMMs plus the two-level output recombination:
inner (L2) U-terms on DVE fused with the PSUM drains, outer (L1) U-terms
on GpSimd from fp16 staging. Staging is fp16 (error budget allows it),
which halves both DVE time and SBUF footprint; output is written fp16 and
upcast on the host.

Winograd scheme per level (products executed M1,M6,M2,M7,M4,M5,M3):
  M1=P11*Q11  M2=P12*Q21  M3=S4*Q22  M4=P22*T4  M5=S1*T1  M6=S2*T2  M7=S3*T3
  C11=M1+M2  U2=M1+M6  U3=U2+M7  U4=U2+M5  C12=U4+M3  C21=U3-M4  C22=U3+M5
The exec order keeps at most 3 staging groups alive per level:
L1 24 x [128,1024] fp16 tiles (48 KB/part), L2 2x12 x [128,512] (24 KB/part).

Per product one 1 MB W blob (ACT HWDGE ring) and one 1 MB Q blob (SP ring),
both contiguous 8 KB/partition, prefetched 2 products ahead; out pieces
(128 KB) ride the SP ring as produced. Product 0's W is host-packed
kt-major and streamed in kt-layer pairs down the ACT ring while its Q
streams in matching kt-chunk pairs down SP — both in exact consumption
order, so the PE tracks the DMA arrival frontier during the ramp; its 4
column sweeps run kt-interleaved across 4 PSUM banks. The final sweep
runs as two m-halves so half X's drain hides under half Y's matmuls.
PSUM banks rotate mod 8; every bank is drained by the fused DVE combo
right after its accumulation stops.
"""

import numpy as np

WORLD = 8
PM, PN = 4, 2          # core grid: 4 M-groups x 2 N-groups
MC = 2048              # per-core M (= rows of C slice)
NCOLS = 2048           # per-core N (= cols of C slice)
K = 4096
NPROD = 49
EXEC = [1, 6, 2, 7, 4, 5, 3]   # Winograd product execution order (both levels)

MM_DTYPE = "float16"


def _lcombos(P):
    # left/weight-side combos of [n, k] matrix quadrants
    n, k = P.shape
    h, g = n // 2, k // 2
    P11, P12 = P[:h, :g], P[:h, g:]
    P21, P22 = P[h:, :g], P[h:, g:]
    S1 = P21 + P22
    S2 = S1 - P11
    S3 = P11 - P21
    S4 = P12 - S2
    return {1: P11, 2: P12, 3: S4, 4: P22, 5: S1, 6: S2, 7: S3}


def _rcombos_mT(Am):
    # right-side combos, m-major (Am = R^T [m, k]); returns RT_p [m/2, k/2]
    m, k = Am.shape
    mh, kh = m // 2, k // 2
    B00, B01 = Am[:mh, :kh], Am[:mh, kh:]
    B10, B11 = Am[mh:, :kh], Am[mh:, kh:]
    return {
        1: B00,
        2: B01,
        3: B11,
        4: B11 - B10 + B00 - B01,
        5: B10 - B00,
        6: B11 - B10 + B00,
        7: B11 - B10,
    }


def _build_nc():
    from contextlib import ExitStack

    from concourse import bacc, mybir, tile

    f32 = mybir.dt.float32
    mm_dt = getattr(mybir.dt, MM_DTYPE)

    nc = bacc.Bacc("TRN2", target_bir_lowering=False, debug=False)

    # product 0 W, kt-major: w1[kp, kt*512 + c*128 + j] = L0[128c+j, 128kt+kp]
    w1_ext = nc.dram_tensor("w1", [128, 4096], mm_dt, kind="ExternalInput")
    # products 1..48, c-major: w[i, kp, c*1024 + kt*128 + j] = L[128c+j, 128kt+kp]
    w_ext = nc.dram_tensor("w", [48, 128, 4096], mm_dt, kind="ExternalInput")
    # rhs blobs: a[i, kp, kt*512 + m] = R[128kt+kp, m]
    a_ext = nc.dram_tensor("a", [NPROD, 128, 4096], mm_dt, kind="ExternalInput")
    # out rows: row = b*16 + t*2 + mh; b in (C11,C12,C21,C22), t n-tile, mh m-half
    out_ext = nc.dram_tensor("out", [64, 128, 512], mm_dt, kind="ExternalOutput")

    with tile.TileContext(nc) as tc, ExitStack() as ctx:
        q_pool = ctx.enter_context(tc.tile_pool(name="q", bufs=4))
        w_pool = ctx.enter_context(tc.tile_pool(name="w", bufs=4))
        w1_pool = ctx.enter_context(tc.tile_pool(name="w1", bufs=1))
        l1_pool = ctx.enter_context(tc.tile_pool(name="l1", bufs=1))
        l2_pool = ctx.enter_context(tc.tile_pool(name="l2", bufs=1))
        mt_pool = ctx.enter_context(tc.tile_pool(name="mt", bufs=8))
        o_pool = ctx.enter_context(tc.tile_pool(name="o", bufs=8))
        s_pool = ctx.enter_context(tc.tile_pool(name="s", bufs=1))
        ps_pool = ctx.enter_context(tc.tile_pool(name="ps", bufs=1, space="PSUM"))

        # ---- PE warmup during the DMA ramp (pulls the HAM clock-gate) ----
        wu = s_pool.tile([128, 128], mm_dt, name="wu", tag="wu")
        nc.vector.memset(wu[:], 0)
        ps_wu = ps_pool.tile([128, 512], f32, name="pswu", tag="b6")
        for _ in range(26):
            nc.tensor.matmul(ps_wu[:, :128], wu[:], wu[:], start=True, stop=True)

        # ---- ramp: product 0's W layers + Q chunks in consumption order ----
        q0 = q_pool.tile([128, 4096], mm_dt, name="q0", tag="q")
        w1 = w1_pool.tile([128, 4096], mm_dt, name="w1", tag="w1")
        # w1 rides the ACT ring in kt-layer pairs; q0 rides SP in kt chunks —
        # both in consumption order, so the two rings split the ramp evenly
        nc.scalar.dma_start(w1[:, 0:1024], w1_ext[:, 0:1024])     # kt layers 0-1
        nc.sync.dma_start(q0[:, 0:1024], a_ext[0, :, 0:1024])     # kt chunks 0-1
        for t in range(1, 4):
            nc.scalar.dma_start(
                w1[:, t * 1024 : (t + 1) * 1024], w1_ext[:, t * 1024 : (t + 1) * 1024]
            )
            nc.sync.dma_start(
                q0[:, t * 1024 : (t + 1) * 1024], a_ext[0, :, t * 1024 : (t + 1) * 1024]
            )

        # ---- streamed W/Q blobs, 2 products ahead ----
        w_bufs, q_bufs = [], []

        def issue_w(idx):
            if idx >= NPROD:
                return
            wt = w_pool.tile([128, 4096], mm_dt, name=f"w{idx}", tag="w")
            nc.scalar.dma_start(wt[:], w_ext[idx - 1])
            w_bufs.append(wt)

        def issue_q(idx):
            if idx >= NPROD:
                return
            qt = q_pool.tile([128, 4096], mm_dt, name=f"q{idx}", tag="q")
            nc.sync.dma_start(qt[:], a_ext[idx])
            q_bufs.append(qt)

        for idx in (1, 2):
            issue_w(idx)
            issue_q(idx)

        # ---- staging ----
        groups = {}  # logical L1 group name -> 8 tiles [128, 1024]

        def new_group(name, tag):
            groups[name] = [
                l1_pool.tile(
                    [128, 1024], mm_dt, name=f"{name}_{t}", tag=f"L{tag}{t}"
                )
                for t in range(8)
            ]

        def l2_group(i, g):
            return [
                l2_pool.tile(
                    [128, 512], mm_dt, name=f"i{i}{g}{c}", tag=f"i{i % 2}{g}{c}"
                )
                for c in range(4)
            ]

        sweep = 0
        m1q = u2q = u3q = None
        u4q = [None] * 4

        def out_piece(i, op, l1t, dst_sl, b, t, mh, mlo, mhi, tmp):
            o = o_pool.tile(
                [128, mhi - mlo], mm_dt, name=f"o{i}_{b}_{t}_{mh}", tag="o"
            )
            g_op = nc.vector.tensor_add if op == "add" else nc.vector.tensor_sub
            g_op(o[:], l1t[:, dst_sl], tmp[:])
            nc.sync.dma_start(out_ext[b * 16 + t * 2 + mh, :, mlo:mhi], o[:])

        def finish(i, c, ps, op, stage_tile, t, mh, mlo, mhi):
            # inner-final piece (quadrant of outer product Mp) + L1-level op
            msl = slice(mlo, mhi)
            dst_sl = slice(mh * 512 + mlo, mh * 512 + mhi)
            vec_op = nc.vector.tensor_add if op == "add" else nc.vector.tensor_sub
            if i == 0:
                vec_op(groups["M1"][t][:, dst_sl], stage_tile[:, msl], ps[:])
                return
            tmp = mt_pool.tile(
                [128, mhi - mlo], mm_dt, name=f"mt{i}_{t}_{mh}", tag="mt"
            )
            vec_op(tmp[:], stage_tile[:, msl], ps[:])
            p = EXEC[i]
            if p == 6:    # U2 = M1 + M6
                nc.vector.tensor_add(
                    groups["U2"][t][:, dst_sl], groups["M1"][t][:, dst_sl], tmp[:]
                )
            elif p == 7:  # U3 = U2 + M7
                nc.vector.tensor_add(
                    groups["U3"][t][:, dst_sl], groups["U2"][t][:, dst_sl], tmp[:]
                )
            elif p == 5:  # U4 = U2 + M5 ; C22 = U3 + M5
                nc.vector.tensor_add(
                    groups["U4"][t][:, dst_sl], groups["U2"][t][:, dst_sl], tmp[:]
                )
                out_piece(i, "add", groups["U3"][t], dst_sl, 3, t, mh, mlo, mhi, tmp)
            elif p == 2:  # C11 = M1 + M2
                out_piece(i, "add", groups["M1"][t], dst_sl, 0, t, mh, mlo, mhi, tmp)
            elif p == 4:  # C21 = U3 - M4
                out_piece(i, "sub", groups["U3"][t], dst_sl, 2, t, mh, mlo, mhi, tmp)
            else:         # p == 3: C12 = U4 + M3
                out_piece(i, "add", groups["U4"][t], dst_sl, 1, t, mh, mlo, mhi, tmp)

        def do_combo(i, j, c, ps, mlo=0, mhi=512):
            msl = slice(mlo, mhi)
            if j == 0:    # m1 of this outer product
                nc.vector.tensor_copy(m1q[c][:, msl], ps[:])
            elif j == 1:  # q=6: u2 = m1 + m6
                nc.vector.tensor_add(u2q[c][:, msl], m1q[c][:, msl], ps[:])
            elif j == 3:  # q=7: u3 = u2 + m7
                nc.vector.tensor_add(u3q[c][:, msl], u2q[c][:, msl], ps[:])
            elif j == 2:  # q=2: quadrant 11 = m1 + m2
                finish(i, c, ps, "add", m1q[c], c, 0, mlo, mhi)
            elif j == 4:  # q=4: quadrant 21 = u3 - m4
                finish(i, c, ps, "sub", u3q[c], 4 + c, 0, mlo, mhi)
            elif j == 5:  # q=5: u4 = u2 + m5 ; quadrant 22 = u3 + m5
                nc.vector.tensor_add(u4q[c][:, msl], u2q[c][:, msl], ps[:])
                finish(i, c, ps, "add", u3q[c], 4 + c, 1, mlo, mhi)
            else:         # j=6, q=3: quadrant 12 = u4 + m3
                finish(i, c, ps, "add", u4q[c], c, 1, mlo, mhi)

        # ---- main loop: 49 products ----
        for i in range(7):
            if i == 0:
                new_group("M1", "A")
            elif i == 1:
                new_group("U2", "B")
            elif i == 3:
                new_group("U3", "C")
            elif i == 5:
                new_group("U4", "A")
            for j in range(7):
                idx = 7 * i + j
                if j == 0:
                    m1q = l2_group(i, "A")
                elif j == 1:
                    u2q = l2_group(i, "B")
                elif j == 3:
                    u3q = l2_group(i, "C")
                elif j == 5:
                    u4q = l2_group(i, "D")

                if idx == 0:
                    # kt-interleaved across 4 banks: PE tracks the DMA frontier
                    psA = [
                        ps_pool.tile([128, 512], f32, name=f"psA{c}", tag=f"b{c}")
                        for c in range(4)
                    ]
                    sweep = 4
                    for kt in range(8):
                        for c in range(4):
                            nc.tensor.matmul(
                                psA[c][:],
                                w1[:, kt * 512 + c * 128 : kt * 512 + (c + 1) * 128],
                                q0[:, kt * 512 : (kt + 1) * 512],
                                start=(kt == 0),
                                stop=(kt == 7),
                            )
                    for c in range(4):
                        do_combo(i, j, c, psA[c])
                    continue

                qt, wt = q_bufs.pop(0), w_bufs.pop(0)
                issue_w(idx + 2)
                issue_q(idx + 2)
                for c in range(4):
                    spans = [(0, 512)]
                    if idx == NPROD - 1 and c == 3:
                        # split the final sweep into m-halves to hide the drain
                        spans = [(0, 256), (256, 512)]
                    for mlo, mhi in spans:
                        ps = ps_pool.tile(
                            [128, mhi - mlo],
                            f32,
                            name=f"ps{idx}_{c}_{mlo}",
                            tag=f"b{sweep % 8}",
                        )
                        sweep += 1
                        for kt in range(8):
                            nc.tensor.matmul(
                                ps[:],
                                wt[:, c * 1024 + kt * 128 : c * 1024 + (kt + 1) * 128],
                                qt[:, kt * 512 + mlo : kt * 512 + mhi],
                                start=(kt == 0),
                                stop=(kt == 7),
                            )
                        do_combo(i, j, c, ps, mlo, mhi)

    nc.compile()
    return nc


def _prep_inputs(A_shards, weight, transed_weight=0):
    np_dt = np.float16

    try:
        transed = bool(int(np.asarray(transed_weight)))
    except (TypeError, ValueError):
        transed = bool(transed_weight)

    Wf = np.asarray(weight, dtype=np.float32)
    Wn = Wf.T if transed else Wf            # [N, K], rows = output columns
    A = np.asarray(A_shards, dtype=np.float32).reshape(WORLD * 1024, K)

    w_blobs = []
    for ng in range(PN):
        P = Wn[ng * NCOLS : (ng + 1) * NCOLS]      # [2048, 4096]
        Lout = _lcombos(P)
        w1 = None
        wrest = np.empty((48, 128, 4096), np_dt)
        for i, p in enumerate(EXEC):
            Lin = _lcombos(Lout[p])
            for j, q in enumerate(EXEC):
                idx = 7 * i + j
                Lc = Lin[q].astype(np_dt)          # [512, 1024]
                t4 = Lc.reshape(4, 128, 8, 128)    # [c, j, kt, kp]
                if idx == 0:
                    w1 = np.ascontiguousarray(
                        t4.transpose(3, 2, 0, 1).reshape(128, 4096)
                    )
                else:
                    wrest[idx - 1] = t4.transpose(3, 0, 2, 1).reshape(128, 4096)
        w_blobs.append((w1, wrest))

    in_maps = []
    for r in range(WORLD):
        mg, ng = r % PM, r // PM
        Am = A[mg * MC : (mg + 1) * MC]            # [2048, 4096] = R^T m-major
        Rout = _rcombos_mT(Am)
        a_blob = np.empty((NPROD, 128, 4096), np_dt)
        for i, p in enumerate(EXEC):
            Rin = _rcombos_mT(Rout[p])
            for j, q in enumerate(EXEC):
                RTc = Rin[q].astype(np_dt)         # [512 m, 1024 k]
                a_blob[7 * i + j] = (
                    RTc.T.reshape(8, 128, 512).transpose(1, 0, 2).reshape(128, 4096)
                )
        w1, wrest = w_blobs[ng]
        in_maps.append({"w1": w1, "w": wrest, "a": a_blob})
    return in_maps


def _gather_output(results):
    C = np.empty((WORLD * 1024, K), np.float32)
    for r in range(WORLD):
        mg, ng = r % PM, r // PM
        o = np.asarray(results[r]["out"], dtype=np.float32).reshape(4, 8, 2, 128, 512)
        CT = np.empty((NCOLS, MC), np.float32)
        for b, (n0, m0) in enumerate([(0, 0), (0, 1024), (1024, 0), (1024, 1024)]):
            # o[b]: [nt, mh, p, m] -> block[n = nt*128+p, m = mh*512+m]
            CT[n0 : n0 + 1024, m0 : m0 + 1024] = (
                o[b].transpose(0, 2, 1, 3).reshape(1024, 1024)
            )
        C[mg * MC : (mg + 1) * MC, ng * NCOLS : (ng + 1) * NCOLS] = CT.T
    return C


_NC = None


def _get_nc():
    global _NC
    if _NC is None:
        _NC = _build_nc()
    return _NC


def kernel(A_shards, weight, transed_weight=0, **_ignored):
    from concourse import bass_utils

    nc = _get_nc()
    in_maps = _prep_inputs(A_shards, weight, transed_weight)
    res = bass_utils.run_bass_kernel_spmd(nc, in_maps, core_ids=list(range(WORLD)))
    return _gather_output(res.results)


if __name__ == "__main__":
    rng = np.random.default_rng(0)
    A = rng.standard_normal((WORLD, 1024, K), dtype=np.float32)
    W = (rng.standard_normal((K, K), dtype=np.float32) * 0.02).astype(np.float32)
    out = kernel(A, W, 0)
    ref = A.reshape(WORLD * 1024, K) @ W.T
    err = np.abs(out - ref).max() / max(np.abs(ref).max(), 1e-12)
    print("abs-rel err vs local numpy:", err)


# revision 5
# speedup vs baseline: 1.0045x; 1.0035x over previous
"""AG-GEMM on 8 TRN2 NeuronCores — 2-level Strassen-Winograd on a 4x2 core grid.

Reference: A_full[8192, 4096] @ weight.T[4096, 4096] -> C[8192, 4096].

Core r = (mg, ng) with mg = r % 4, ng = r // 4 computes the output slice
C[mg*2048:(mg+1)*2048, ng*2048:(ng+1)*2048] as CT = P @ Q with
P = W_rows[ng] [2048 n, 4096 k] and Q = A_rows[mg]^T [4096 k, 2048 m]
(weight row-slices replicated across the 4 cores of an n-group; no
collective). The 4x2 grid keeps per-core M at 2048 so that after TWO
Strassen-Winograd levels the matmul moving-dim is still 512 (the PSUM-bank
sweet spot). 49 quarter-products [512n,1024k]@[1024k,512m] replace 64:
PE work drops to 49/64 of dense (1568 vs 2048 128x512 matmuls).

All operand-side combinations (both levels) are precomputed on the host.
The device runs the 49 GEMMs plus the two-level output recombination:
inner (L2) U-terms on DVE fused with the PSUM drains, outer (L1) U-terms
on GpSimd from fp16 staging. Staging is fp16 (error budget allows it),
which halves both DVE time and SBUF footprint; output is written fp16 and
upcast on the host.

Winograd scheme per level (products executed M1,M6,M2,M7,M4,M5,M3):
  M1=P11*Q11  M2=P12*Q21  M3=S4*Q22  M4=P22*T4  M5=S1*T1  M6=S2*T2  M7=S3*T3
  C11=M1+M2  U2=M1+M6  U3=U2+M7  U4=U2+M5  C12=U4+M3  C21=U3-M4  C22=U3+M5
The exec order keeps at most 3 staging groups alive per level:
L1 24 x [128,1024] fp16 tiles (48 KB/part), L2 2x12 x [128,512] (24 KB/part).

Per product one 1 MB W blob (ACT HWDGE ring) and one 1 MB Q blob (SP ring),
both contiguous 8 KB/partition, prefetched 2 products ahead; out pieces
(128 KB) ride the SP ring as produced. Product 0's W is host-packed
kt-major and interleaved with its Q chunks in exact consumption order so
the PE tracks the DMA arrival frontier during the ramp (first 2 layers on
the otherwise-idle ACT ring); its 4 column sweeps run kt-interleaved
across 4 PSUM banks. The final sweep runs as two m-halves so half X's
drain hides under half Y's matmuls. PSUM banks rotate mod 8; every bank
is drained by the fused DVE combo right after its accumulation stops.
"""

import numpy as np

WORLD = 8
PM, PN = 4, 2          # core grid: 4 M-groups x 2 N-groups
MC = 2048              # per-core M (= rows of C slice)
NCOLS = 2048           # per-core N (= cols of C slice)
K = 4096
NPROD = 49
EXEC = [1, 6, 2, 7, 4, 5, 3]   # Winograd product execution order (both levels)

MM_DTYPE = "float16"


def _lcombos(P):
    # left/weight-side combos of [n, k] matrix quadrants
    n, k = P.shape
    h, g = n // 2, k // 2
    P11, P12 = P[:h, :g], P[:h, g:]
    P21, P22 = P[h:, :g], P[h:, g:]
    S1 = P21 + P22
    S2 = S1 - P11
    S3 = P11 - P21
    S4 = P12 - S2
    return {1: P11, 2: P12, 3: S4, 4: P22, 5: S1, 6: S2, 7: S3}


def _rcombos_mT(Am):
    # right-side combos, m-major (Am = R^T [m, k]); returns RT_p [m/2, k/2]
    m, k = Am.shape
    mh, kh = m // 2, k // 2
    B00, B01 = Am[:mh, :kh], Am[:mh, kh:]
    B10, B11 = Am[mh:, :kh], Am[mh:, kh:]
    return {
        1: B00,
        2: B01,
        3: B11,
        4: B11 - B10 + B00 - B01,
        5: B10 - B00,
        6: B11 - B10 + B00,
        7: B11 - B10,
    }


def _build_nc():
    from contextlib import ExitStack

    from concourse import bacc, mybir, tile

    f32 = mybir.dt.float32
    mm_dt = getattr(mybir.dt, MM_DTYPE)

    nc = bacc.Bacc("TRN2", target_bir_lowering=False, debug=False)

    # product 0 W, kt-major: w1[kp, kt*512 + c*128 + j] = L0[128c+j, 128kt+kp]
    w1_ext = nc.dram_tensor("w1", [128, 4096], mm_dt, kind="ExternalInput")
    # products 1..48, c-major: w[i, kp, c*1024 + kt*128 + j] = L[128c+j, 128kt+kp]
    w_ext = nc.dram_tensor("w", [48, 128, 4096], mm_dt, kind="ExternalInput")
    # rhs blobs: a[i, kp, kt*512 + m] = R[128kt+kp, m]
    a_ext = nc.dram_tensor("a", [NPROD, 128, 4096], mm_dt, kind="ExternalInput")
    # out rows: row = b*16 + t*2 + mh; b in (C11,C12,C21,C22), t n-tile, mh m-half
    out_ext = nc.dram_tensor("out", [64, 128, 512], mm_dt, kind="ExternalOutput")

    with tile.TileContext(nc) as tc, ExitStack() as ctx:
        q_pool = ctx.enter_context(tc.tile_pool(name="q", bufs=4))
        w_pool = ctx.enter_context(tc.tile_pool(name="w", bufs=4))
        w1_pool = ctx.enter_context(tc.tile_pool(name="w1", bufs=1))
        l1_pool = ctx.enter_context(tc.tile_pool(name="l1", bufs=1))
        l2_pool = ctx.enter_context(tc.tile_pool(name="l2", bufs=1))
        mt_pool = ctx.enter_context(tc.tile_pool(name="mt", bufs=8))
        o_pool = ctx.enter_context(tc.tile_pool(name="o", bufs=8))
        s_pool = ctx.enter_context(tc.tile_pool(name="s", bufs=1))
        ps_pool = ctx.enter_context(tc.tile_pool(name="ps", bufs=1, space="PSUM"))

        # ---- PE warmup during the DMA ramp (pulls the HAM clock-gate) ----
        wu = s_pool.tile([128, 128], mm_dt, name="wu", tag="wu")
        nc.vector.memset(wu[:], 0)
        ps_wu = ps_pool.tile([128, 512], f32, name="pswu", tag="b6")
        for _ in range(26):
            nc.tensor.matmul(ps_wu[:, :128], wu[:], wu[:], start=True, stop=True)

        # ---- ramp: product 0's W layers + Q chunks in consumption order ----
        q0 = q_pool.tile([128, 4096], mm_dt, name="q0", tag="q")
        w1 = w1_pool.tile([128, 4096], mm_dt, name="w1", tag="w1")
        # w1 rides the ACT ring in kt-layer pairs; q0 rides SP in kt chunks —
        # both in consumption order, so the two rings split the ramp evenly
        nc.scalar.dma_start(w1[:, 0:1024], w1_ext[:, 0:1024])     # kt layers 0-1
        nc.sync.dma_start(q0[:, 0:1024], a_ext[0, :, 0:1024])     # kt chunks 0-1
        for t in range(1, 4):
            nc.scalar.dma_start(
                w1[:, t * 1024 : (t + 1) * 1024], w1_ext[:, t * 1024 : (t + 1) * 1024]
            )
            nc.sync.dma_start(
                q0[:, t * 1024 : (t + 1) * 1024], a_ext[0, :, t * 1024 : (t + 1) * 1024]
            )

        # ---- streamed W/Q blobs, 2 products ahead ----
        w_bufs, q_bufs = [], []

        def issue_w(idx):
            if idx >= NPROD:
                return
            wt = w_pool.tile([128, 4096], mm_dt, name=f"w{idx}", tag="w")
            nc.scalar.dma_start(wt[:], w_ext[idx - 1])
            w_bufs.append(wt)

        def issue_q(idx):
            if idx >= NPROD:
                return
            qt = q_pool.tile([128, 4096], mm_dt, name=f"q{idx}", tag="q")
            nc.sync.dma_start(qt[:], a_ext[idx])
            q_bufs.append(qt)

        for idx in (1, 2):
            issue_w(idx)
            issue_q(idx)

        # ---- staging ----
        groups = {}  # logical L1 group name -> 8 tiles [128, 1024]

        def new_group(name, tag):
            groups[name] = [
                l1_pool.tile(
                    [128, 1024], mm_dt, name=f"{name}_{t}", tag=f"L{tag}{t}"
                )
                for t in range(8)
            ]

        def l2_group(i, g):
            return [
                l2_pool.tile(
                    [128, 512], mm_dt, name=f"i{i}{g}{c}", tag=f"i{i % 2}{g}{c}"
                )
                for c in range(4)
            ]

        sweep = 0
        m1q = u2q = u3q = None
        u4q = [None] * 4
        pre = [None] * 4

        def out_piece(i, op, l1t, dst_sl, b, t, mh, mlo, mhi, tmp):
            o = o_pool.tile(
                [128, mhi - mlo], mm_dt, name=f"o{i}_{b}_{t}_{mh}", tag="o"
            )
            g_op = nc.vector.tensor_add if op == "add" else nc.vector.tensor_sub
            g_op(o[:], l1t[:, dst_sl], tmp[:])
            nc.sync.dma_start(out_ext[b * 16 + t * 2 + mh, :, mlo:mhi], o[:])

        def finish(i, c, ps, op, stage_tile, t, mh, mlo, mhi):
            # inner-final piece (quadrant of outer product Mp) + L1-level op
            msl = slice(mlo, mhi)
            dst_sl = slice(mh * 512 + mlo, mh * 512 + mhi)
            vec_op = nc.vector.tensor_add if op == "add" else nc.vector.tensor_sub
            if i == 0:
                vec_op(groups["M1"][t][:, dst_sl], stage_tile[:, msl], ps[:])
                return
            tmp = mt_pool.tile(
                [128, mhi - mlo], mm_dt, name=f"mt{i}_{t}_{mh}", tag="mt"
            )
            vec_op(tmp[:], stage_tile[:, msl], ps[:])
            p = EXEC[i]
            if p == 6:    # U2 = M1 + M6
                nc.vector.tensor_add(
                    groups["U2"][t][:, dst_sl], groups["M1"][t][:, dst_sl], tmp[:]
                )
            elif p == 7:  # U3 = U2 + M7
                nc.vector.tensor_add(
                    groups["U3"][t][:, dst_sl], groups["U2"][t][:, dst_sl], tmp[:]
                )
            elif p == 5:  # U4 = U2 + M5 ; C22 = U3 + M5
                nc.vector.tensor_add(
                    groups["U4"][t][:, dst_sl], groups["U2"][t][:, dst_sl], tmp[:]
                )
                out_piece(i, "add", groups["U3"][t], dst_sl, 3, t, mh, mlo, mhi, tmp)
            elif p == 2:  # C11 = M1 + M2
                out_piece(i, "add", groups["M1"][t], dst_sl, 0, t, mh, mlo, mhi, tmp)
            elif p == 4:  # C21 = U3 - M4
                out_piece(i, "sub", groups["U3"][t], dst_sl, 2, t, mh, mlo, mhi, tmp)
            else:         # p == 3: C12 = U4 + M3
                out_piece(i, "add", groups["U4"][t], dst_sl, 1, t, mh, mlo, mhi, tmp)

        def do_combo(i, j, c, ps, mlo=0, mhi=512):
            msl = slice(mlo, mhi)
            if j == 0:    # m1 of this outer product
                nc.vector.tensor_copy(m1q[c][:, msl], ps[:])
            elif j == 1:  # q=6: u2 = m1 + m6
                nc.vector.tensor_add(u2q[c][:, msl], m1q[c][:, msl], ps[:])
            elif j == 3:  # q=7: u3 = u2 + m7
                nc.vector.tensor_add(u3q[c][:, msl], u2q[c][:, msl], ps[:])
            elif j == 2:  # q=2: quadrant 11 = m1 + m2
                finish(i, c, ps, "add", m1q[c], c, 0, mlo, mhi)
            elif j == 4:  # q=4: quadrant 21 = u3 - m4
                finish(i, c, ps, "sub", u3q[c], 4 + c, 0, mlo, mhi)
            elif j == 5:  # q=5: u4 = u2 + m5 ; quadrant 22 = u3 + m5
                nc.vector.tensor_add(u4q[c][:, msl], u2q[c][:, msl], ps[:])
                finish(i, c, ps, "add", u3q[c], 4 + c, 1, mlo, mhi)
                if i == 6:
                    # last outer product: pre-add U4 + u4 now, while the PE
                    # streams, so each final C12 piece needs only ONE op
                    # after its psum stops — shortens the end-of-kernel tail
                    pre[c] = l2_pool.tile(
                        [128, 512], mm_dt, name=f"pre{c}", tag=f"pr{c}"
                    )
                    nc.vector.tensor_add(
                        pre[c][:], groups["U4"][c][:, 512:1024], u4q[c][:]
                    )
            else:         # j=6, q=3: quadrant 12 = u4 + m3
                if i == 6:
                    o = o_pool.tile(
                        [128, mhi - mlo], mm_dt, name=f"of{c}_{mlo}", tag="o"
                    )
                    nc.vector.tensor_add(o[:], pre[c][:, msl], ps[:])
                    nc.sync.dma_start(out_ext[16 + c * 2 + 1, :, mlo:mhi], o[:])
                else:
                    finish(i, c, ps, "add", u4q[c], c, 1, mlo, mhi)

        # ---- main loop: 49 products ----
        for i in range(7):
            if i == 0:
                new_group("M1", "A")
            elif i == 1:
                new_group("U2", "B")
            elif i == 3:
                new_group("U3", "C")
            elif i == 5:
                new_group("U4", "A")
            for j in range(7):
                idx = 7 * i + j
                if j == 0:
                    m1q = l2_group(i, "A")
                elif j == 1:
                    u2q = l2_group(i, "B")
                elif j == 3:
                    u3q = l2_group(i, "C")
                elif j == 5:
                    u4q = l2_group(i, "D")

                if idx == 0:
                    # kt-interleaved across 4 banks: PE tracks the DMA frontier
                    psA = [
                        ps_pool.tile([128, 512], f32, name=f"psA{c}", tag=f"b{c}")
                        for c in range(4)
                    ]
                    sweep = 4
                    for kt in range(8):
                        for c in range(4):
                            nc.tensor.matmul(
                                psA[c][:],
                                w1[:, kt * 512 + c * 128 : kt * 512 + (c + 1) * 128],
                                q0[:, kt * 512 : (kt + 1) * 512],
                                start=(kt == 0),
                                stop=(kt == 7),
                            )
                    for c in range(4):
                        do_combo(i, j, c, psA[c])
                    continue

                qt, wt = q_bufs.pop(0), w_bufs.pop(0)
                issue_w(idx + 2)
                issue_q(idx + 2)
                for c in range(4):
                    spans = [(0, 512)]
                    if idx == NPROD - 1 and c == 3:
                        # split the final sweep into m-halves to hide the drain
                        spans = [(0, 256), (256, 512)]
                    for mlo, mhi in spans:
                        ps = ps_pool.tile(
                            [128, mhi - mlo],
                            f32,
                            name=f"ps{idx}_{c}_{mlo}",
                            tag=f"b{sweep % 8}",
                        )
                        sweep += 1
                        for kt in range(8):
                            nc.tensor.matmul(
                                ps[:],
                                wt[:, c * 1024 + kt * 128 : c * 1024 + (kt + 1) * 128],
                                qt[:, kt * 512 + mlo : kt * 512 + mhi],
                                start=(kt == 0),
                                stop=(kt == 7),
                            )
                        do_combo(i, j, c, ps, mlo, mhi)

    nc.compile()
    return nc


def _prep_inputs(A_shards, weight, transed_weight=0):
    np_dt = np.float16

    try:
        transed = bool(int(np.asarray(transed_weight)))
    except (TypeError, ValueError):
        transed = bool(transed_weight)

    Wf = np.asarray(weight, dtype=np.float32)
    Wn = Wf.T if transed else Wf            # [N, K], rows = output columns
    A = np.asarray(A_shards, dtype=np.float32).reshape(WORLD * 1024, K)

    w_blobs = []
    for ng in range(PN):
        P = Wn[ng * NCOLS : (ng + 1) * NCOLS]      # [2048, 4096]
        Lout = _lcombos(P)
        w1 = None
        wrest = np.empty((48, 128, 4096), np_dt)
        for i, p in enumerate(EXEC):
            Lin = _lcombos(Lout[p])
            for j, q in enumerate(EXEC):
                idx = 7 * i + j
                Lc = Lin[q].astype(np_dt)          # [512, 1024]
                t4 = Lc.reshape(4, 128, 8, 128)    # [c, j, kt, kp]
                if idx == 0:
                    w1 = np.ascontiguousarray(
                        t4.transpose(3, 2, 0, 1).reshape(128, 4096)
                    )
                else:
                    wrest[idx - 1] = t4.transpose(3, 0, 2, 1).reshape(128, 4096)
        w_blobs.append((w1, wrest))

    in_maps = []
    for r in range(WORLD):
        mg, ng = r % PM, r // PM
        Am = A[mg * MC : (mg + 1) * MC]            # [2048, 4096] = R^T m-major
        Rout = _rcombos_mT(Am)
        a_blob = np.empty((NPROD, 128, 4096), np_dt)
        for i, p in enumerate(EXEC):
            Rin = _rcombos_mT(Rout[p])
            for j, q in enumerate(EXEC):
                RTc = Rin[q].astype(np_dt)         # [512 m, 1024 k]
                a_blob[7 * i + j] = (
                    RTc.T.reshape(8, 128, 512).transpose(1, 0, 2).reshape(128, 4096)
                )
        w1, wrest = w_blobs[ng]
        in_maps.append({"w1": w1, "w": wrest, "a": a_blob})
    return in_maps


def _gather_output(results):
    C = np.empty((WORLD * 1024, K), np.float32)
    for r in range(WORLD):
        mg, ng = r % PM, r // PM
        o = np.asarray(results[r]["out"], dtype=np.float32).reshape(4, 8, 2, 128, 512)
        CT = np.empty((NCOLS, MC), np.float32)
        for b, (n0, m0) in enumerate([(0, 0), (0, 1024), (1024, 0), (1024, 1024)]):
            # o[b]: [nt, mh, p, m] -> block[n = nt*128+p, m = mh*512+m]
            CT[n0 : n0 + 1024, m0 : m0 + 1024] = (
                o[b].transpose(0, 2, 1, 3).reshape(1024, 1024)
            )
        C[mg * MC : (mg + 1) * MC, ng * NCOLS : (ng + 1) * NCOLS] = CT.T
    return C


_NC = None


def _get_nc():
    global _NC
    if _NC is None:
        _NC = _build_nc()
    return _NC


def kernel(A_shards, weight, transed_weight=0, **_ignored):
    from concourse import bass_utils

    nc = _get_nc()
    in_maps = _prep_inputs(A_shards, weight, transed_weight)
    res = bass_utils.run_bass_kernel_spmd(nc, in_maps, core_ids=list(range(WORLD)))
    return _gather_output(res.results)


if __name__ == "__main__":
    rng = np.random.default_rng(0)
    A = rng.standard_normal((WORLD, 1024, K), dtype=np.float32)
    W = (rng.standard_normal((K, K), dtype=np.float32) * 0.02).astype(np.float32)
    out = kernel(A, W, 0)
    ref = A.reshape(WORLD * 1024, K) @ W.T
    err = np.abs(out - ref).max() / max(np.abs(ref).max(), 1e-12)
    print("abs-rel err vs local numpy:", err)
